# revision 24
# baseline (speedup 1.0000x reference)
"""Trainium2 Bass kernel for a 6-layer transformer decoder (B=8, S=512, D=512,
H=8, DK=DV=64, DFF=2048, vocab 32000).

Strategy: data-parallel over the batch — each of the 8 NeuronCores runs the
full decoder stack for one batch element. No collectives needed.

On-device layout: activations are kept transposed, xT[d, s], stored as SBUF
tiles [128, 4, 512] (partition = d % 128, then d-subtile, then s). Matmuls run
on the PE in bf16 (fp32 PSUM accumulation).

v2 scheduling notes (engine queues are per-engine FIFO in emission order):
 - Attention emits ALL pairs' score matmuls before any AV matmul so the PE
   queue stays dense while ACT chews softmax exps (avoids HAM re-throttle to
   1.2 GHz, which made AV/score matmuls run at half clock in v1).
 - Cross-attention K/V projections (independent of self-attn) are emitted
   interleaved between self-attn AV pairs as PE filler.
 - Softmax denominator: ones-column in augmented V gives the denominator row
   in PSUM; DVE reciprocal_approx_fast reads it straight from PSUM, GPSIMD
   partition_broadcast (otherwise-idle engine) spreads it across 64
   partitions, one DVE multiply applies it during the PSUM->SBUF eviction.
   (v1 used 2 ACT copies + a K=1 broadcast matmul per head.)
 - LayerNorm rsd = exp(-0.5*ln(var)) so every ACT function used (exp, ln,
   relu, identity, copy) lives in the single `natural_log_exp_and_others`
   table set -> no ACT_TABLE_LOAD thrash (v1 paid 24 loads for Sqrt).
 - All-zero biases / unit LN gains (true for this model) are detected on the
   host and compiled out; general fallback paths remain.
"""

import os
import numpy as np

_CONCOURSE_PATHS = ["/opt/trn_rl_repo", "/root/.axon_site/_ro/trn_rl_repo"]


def _ensure_path():
    try:
        import concourse.bass  # noqa: F401
    except Exception:
        import sys

        for p in _CONCOURSE_PATHS:
            if p not in sys.path and os.path.isdir(p):
                sys.path.insert(0, p)


V, D, NL, DK, DVh, H, DFF = 32000, 512, 6, 64, 64, 8, 2048
B, S = 8, 512
EPS = 1e-5
P = 128
NSUB = D // P  # 4 d-subtiles
NCH = S // P  # 4 s-chunks
NF = DFF // P  # 16 dff-chunks
HW_COLS = H * (DVh + 1)  # 520 augmented-v columns

# Debug knobs (test.py may override before calling kernel()).
N_LAYERS = NL
TAPS = ()  # e.g. ("sa0", "x1_0", "ca0", "x2_0", "ff0")
MM_DT = "bf16"  # "f32r" | "bf16"

# Results of the last kernel() call (for test.py).
LAST_RESULT = None

_BUILD_CACHE = {}


def _pe_table():
    pos = np.arange(S)[:, None].astype(np.float32)
    i = np.arange(0, D, 2).astype(np.float32)
    ang = pos / np.power(10000.0, i / D)
    pe = np.zeros((S, D), dtype=np.float32)
    pe[:, 0::2] = np.sin(ang)
    pe[:, 1::2] = np.cos(ang)
    return pe


def _to_T_tiles(mat):
    """[S, D]-like -> [P, NSUB, S] transposed-tile layout (mat.T chunked)."""
    t = np.ascontiguousarray(np.asarray(mat, np.float32)).T  # [D, S]
    return np.ascontiguousarray(t.reshape(t.shape[0] // P, P, -1).transpose(1, 0, 2))


def _col_layout(vec):
    """[D]-like -> [P, D//P] per-partition column layout."""
    v = np.asarray(vec, np.float32).reshape(-1)
    return np.ascontiguousarray(v.reshape(v.shape[0] // P, P).T)


def _act_table_patch():
    """Restrict the ACT table-set chooser to `natural_log_exp_and_others`
    (contains every function this kernel uses: exp, ln, relu, identity,
    copy) so exactly one ACT_TABLE_LOAD is emitted. The default chooser
    maps Exp -> exp_and_others but Ln -> natural_log, inserting two ~1.3us
    table loads inside every LayerNorm. Keys/order are preserved so the
    act_func_set_id indices stay valid; only the *membership* used by the
    chooser is emptied for the other sets. Returns an undo callable."""
    import concourse.bacc as bacc_mod
    import concourse.bass_interp as interp_mod

    keep = "natural_log_exp_and_others"
    orig = bacc_mod.get_activation_tables

    def only_keep(arch):
        tabs = orig(arch)
        return {k: (v if k == keep else set()) for k, v in tabs.items()}

    patched = []
    for mod in (bacc_mod, interp_mod):
        if getattr(mod, "get_activation_tables", None) is not None:
            patched.append((mod, mod.get_activation_tables))
            mod.get_activation_tables = only_keep

    def undo():
        for mod, fn in patched:
            mod.get_activation_tables = fn

    return undo


def _build(n_layers, causal_self, self_needs_mask, cross_needs_mask,
           zb_attn, zb_ffn, unit_ln, taps, mm_dt):
    _ensure_path()
    import concourse.mybir as mybir
    from concourse import bacc
    from concourse.tile import TileContext

    dt = mybir.dt
    AF = mybir.ActivationFunctionType
    OP = mybir.AluOpType
    f32 = dt.float32
    fsb = dt.float32r if mm_dt == "f32r" else dt.bfloat16
    # below 256 moving cols fp32r drops to 1/4 rate; bf16 doesn't
    n_floor = 256 if mm_dt == "f32r" else 0

    _undo_act_patch = _act_table_patch()

    nc = bacc.Bacc("TRN2", target_bir_lowering=False, debug=False, num_devices=8)

    def din(name, shape, d=None):
        return nc.dram_tensor(name, shape, d or fsb, kind="ExternalInput")

    x0T_d = din("x0T", [P, NSUB, S])
    peT_d = din("peT", [P, NSUB, S])
    encT_d = din("encT", [P, NSUB, S])
    ones_d = din("ones_row", [1, S])
    invD_d = din("invD_col", [P, 1])
    tri_d = din("tri01", [P, P]) if causal_self else None
    ident_d = din("ident", [P, P]) if (self_needs_mask or cross_needs_mask) else None
    smask_d = din("smaskT8", [P, NCH, S]) if self_needs_mask else None
    cmask_d = din("cmaskT8", [P, NCH, S]) if cross_needs_mask else None

    wq_s_d = din("wq_s", [n_layers, P, NSUB, D])
    wk_s_d = din("wk_s", [n_layers, P, NSUB, D])
    wv_s_d = din("wv_s", [n_layers, P, NSUB, HW_COLS])
    wq_c_d = din("wq_c", [n_layers, P, NSUB, D])
    wk_c_d = din("wk_c", [n_layers, P, NSUB, D])
    wv_c_d = din("wv_c", [n_layers, P, NSUB, HW_COLS])
    if not zb_attn:
        bq_s_d = din("bq_s", [n_layers, P, NSUB], f32)
        bk_s_d = din("bk_s", [n_layers, P, NSUB], f32)
        bv_s_d = din("bv_s", [n_layers, 1, HW_COLS])
        bq_c_d = din("bq_c", [n_layers, P, NSUB], f32)
        bk_c_d = din("bk_c", [n_layers, P, NSUB], f32)
        bv_c_d = din("bv_c", [n_layers, 1, HW_COLS])
    w1_d = din("w1", [n_layers, P, NSUB, DFF])
    w2_d = din("w2", [n_layers, P, NF, D])
    if not zb_ffn:
        b1_d = din("b1c", [n_layers, P, NF], f32)
        b2_d = din("b2c", [n_layers, P, NSUB], f32)
    if not unit_ln:
        ln1g_d = din("ln1g", [n_layers, P, NSUB], f32)
        ln1b_d = din("ln1b", [n_layers, P, NSUB], f32)
        ln2g_d = din("ln2g", [n_layers, P, NSUB], f32)
        ln2b_d = din("ln2b", [n_layers, P, NSUB], f32)

    out_d = nc.dram_tensor("out_xT", [P, NSUB, S], f32, kind="ExternalOutput")
    tap_d = {
        t: nc.dram_tensor(f"tap_{t}", [P, NSUB, S], fsb, kind="ExternalOutput")
        for t in taps
    }

    def mm(out, lhsT, rhs, start, stop):
        nc.tensor.matmul(
            out, lhsT, rhs, start=start, stop=stop, skip_group_check=True
        )

    with TileContext(nc) as tc:
        with (
            nc.allow_low_precision(reason="reduced-precision matmul pipeline"),
            tc.tile_pool(name="wts", bufs=3 if mm_dt == "f32r" else 6) as wpool,
            tc.tile_pool(name="small", bufs=14) as spool,
            tc.tile_pool(name="brows", bufs=2) as brpool,
            tc.tile_pool(name="qk", bufs=3 if mm_dt == "f32r" else 4) as qkpool,
            tc.tile_pool(name="v", bufs=2 if mm_dt == "f32r" else 3) as vpool,
            tc.tile_pool(name="exp", bufs=2 if mm_dt == "f32r" else 4) as epool,
            tc.tile_pool(name="attn", bufs=2 if mm_dt == "f32r" else 3) as apool,
            tc.tile_pool(name="x", bufs=2 if mm_dt == "f32r" else 3) as xpool,
            tc.tile_pool(name="xout", bufs=1) as xopool,
            tc.tile_pool(name="lnt", bufs=3) as tpool,
            tc.tile_pool(name="ff", bufs=1) as ffpool,
            tc.tile_pool(name="row", bufs=8) as rpool,
            tc.tile_pool(name="rb", bufs=4) as rbpool,
            tc.tile_pool(name="const", bufs=1) as cpool,
            tc.tile_pool(name="ps", bufs=8, space="PSUM") as pspool,
        ):
            # ---- constants & persistent activations ----
            ones_sb = cpool.tile([1, S], fsb, tag="c_ones")
            nc.sync.dma_start(ones_sb[:], ones_d[:])
            invD_sb = cpool.tile([P, 1], fsb, tag="c_invD")
            nc.sync.dma_start(invD_sb[:], invD_d[:])
            if causal_self:
                tri_sb = cpool.tile([P, P], fsb, tag="c_tri")
                nc.sync.dma_start(tri_sb[:], tri_d[:])
            if ident_d is not None:
                id_sb = cpool.tile([P, P], fsb, tag="c_id")
                nc.sync.dma_start(id_sb[:], ident_d[:])
            smask_sb = None
            if self_needs_mask:
                smask_sb = cpool.tile([P, NCH, S], fsb, tag="c_smask")
                nc.sync.dma_start(smask_sb[:], smask_d[:])
            cmask_sb = None
            if cross_needs_mask:
                cmask_sb = cpool.tile([P, NCH, S], fsb, tag="c_cmask")
                nc.sync.dma_start(cmask_sb[:], cmask_d[:])

            encT = cpool.tile([P, NSUB, S], fsb, tag="c_enc")
            nc.sync.dma_start(encT[:], encT_d[:])

            # x0 = emb rows (host-gathered) + positional encoding
            x0r = xpool.tile([P, NSUB, S], fsb, tag="x")
            nc.sync.dma_start(x0r[:], x0T_d[:])
            peT_sb = qkpool.tile([P, NSUB, S], fsb, tag="qk")
            nc.sync.dma_start(peT_sb[:], peT_d[:])
            xT = xpool.tile([P, NSUB, S], fsb, tag="x")
            for i in range(NSUB):
                nc.vector.tensor_tensor(
                    xT[:, i, :], x0r[:, i, :], peT_sb[:, i, :], OP.add
                )

            _psn = [0]

            def ps_tile(n=S, p=P):
                _psn[0] += 1
                return pspool.tile([p, n], f32, tag="ps", name=f"ps{_psn[0]}")

            def proj_chunk(t, w_sb, b_sb, srcT, j, on_act):
                """One 128-row output chunk of a dk_all x S projection."""
                ps = ps_tile()
                for i in range(NSUB):
                    mm(
                        ps[:],
                        w_sb[:, i, j * P : (j + 1) * P],
                        srcT[:, i, :],
                        start=(i == 0),
                        stop=(i == NSUB - 1),
                    )
                if b_sb is None:
                    if on_act:
                        nc.scalar.activation(t[:, j, :], ps[:], AF.Identity)
                    else:
                        nc.vector.tensor_copy(t[:, j, :], ps[:])
                elif on_act:
                    nc.scalar.activation(
                        t[:, j, :], ps[:], AF.Identity, bias=b_sb[:, j : j + 1]
                    )
                else:
                    nc.vector.tensor_scalar(
                        t[:, j, :], ps[:], b_sb[:, j : j + 1], None, OP.add
                    )

            def proj_T(w_sb, b_sb, srcT, on_act, chunks=range(NSUB), t=None):
                """dk_all x S projection, transposed output [P, NSUB, S].
                Chunk j is only needed by attention pair j, so later chunks
                can be deferred into the attention AV phase as PE filler."""
                if t is None:
                    t = qkpool.tile([P, NSUB, S], fsb, tag="qk")
                for j in chunks:
                    proj_chunk(t, w_sb, b_sb, srcT, j, on_act)
                return t

            def v_aug_into(vt, w_sb, brow_sb, srcT, sc_list):
                """augmented v chunks, natural orientation: [P(s), NCH, 8, 65].

                Per head 64 value columns + a denominator ones column. The
                ones columns are memset directly (no bias-row matmul) when
                biases are zero; otherwise the K=1 bias-row matmul supplies
                bias + ones as in v1. Split by sc chunks so halves can be
                emitted as PE filler inside LayerNorm chains."""
                half = HW_COLS // 2  # 260
                for sc in sc_list:
                    for hh in range(2):
                        cs, ce = hh * half, (hh + 1) * half
                        ps = ps_tile(n=half)
                        for i in range(NSUB):
                            mm(
                                ps[:],
                                srcT[:, i, sc * P : (sc + 1) * P],
                                w_sb[:, i, cs:ce],
                                start=(i == 0),
                                stop=(brow_sb is None and i == NSUB - 1),
                            )
                        if brow_sb is not None:
                            mm(
                                ps[:],
                                ones_sb[0:1, 0:P],
                                brow_sb[0:1, cs:ce],
                                start=False,
                                stop=True,
                            )
                        nc.vector.tensor_copy(
                            vt[:, sc, hh * 4 : (hh + 1) * 4, :], ps[:]
                        )
                    if brow_sb is None:
                        # overwrite the 8 denominator columns with ones
                        nc.vector.memset(vt[:, sc, :, DVh : DVh + 1], 1.0)

            def v_aug(w_sb, brow_sb, srcT):
                vt = vpool.tile([P, NCH, H, DVh + 1], fsb, tag="v")
                v_aug_into(vt, w_sb, brow_sb, srcT, range(NCH))
                return vt

            def attn_pair_scores(j, qT, kT, causal, mask_sb):
                """Score matmuls + exp + causal mask for one head pair.
                Returns [ex_u0, ex_u1] SBUF tiles."""
                exs = [
                    epool.tile([P, NCH, S], fsb, tag="exp", name=f"ex{j}_0"),
                    epool.tile([P, NCH, S], fsb, tag="exp", name=f"ex{j}_1"),
                ]
                for c in range(NCH):
                    q0 = c * P if causal else 0
                    qs = min(q0, S - n_floor) if causal else 0
                    scs = [ps_tile(), ps_tile()]
                    for u in range(2):
                        ph = u * 64
                        mm(
                            scs[u][:, qs:S],
                            kT[ph : ph + 64, j, c * P : (c + 1) * P],
                            qT[ph : ph + 64, j, qs:S],
                            start=True,
                            stop=(mask_sb is None),
                        )
                        if mask_sb is not None:
                            mm(
                                scs[u][:, qs:S],
                                id_sb[:],
                                mask_sb[:, c, qs:S],
                                start=False,
                                stop=True,
                            )
                    for u in range(2):
                        nc.scalar.activation(
                            exs[u][:, c, q0:S], scs[u][:, q0:S], AF.Exp,
                            scale=0.125,
                        )
                        if causal:
                            nc.vector.tensor_tensor(
                                exs[u][:, c, c * P : (c + 1) * P],
                                exs[u][:, c, c * P : (c + 1) * P],
                                tri_sb[:],
                                OP.mult,
                            )
                return exs

            def attn_av_pair(j, exs, vt, attnT, causal):
                """Phase B for one pair: AV accumulation + denominator chain.

                Denominator row (partition 64 of the AV psum, from the ones
                column in vt) -> DVE reciprocal straight from PSUM -> GPSIMD
                partition_broadcast across 64 partitions -> DVE multiply as
                the PSUM->SBUF eviction."""
                avs = [ps_tile(), ps_tile()]
                for c in range(NCH):
                    q0 = c * P if causal else 0
                    for u in range(2):
                        h = 2 * j + u
                        mm(
                            avs[u][0:65, q0:S],
                            vt[:, c, h, :],
                            exs[u][:, c, q0:S],
                            start=(c == 0),
                            stop=(c == NCH - 1),
                        )
                rcps, rbs = [], []
                for u in range(2):
                    # custom-DVE recip mis-reads PSUM at partition offset 64;
                    # stage the denominator row through SBUF via ACT (the
                    # AV-completion wait must NOT sit on the DVE FIFO — v6
                    # measured +230us when it did)
                    rsu = rpool.tile([1, S], f32, tag="row", name=f"rs{j}_{u}")
                    nc.scalar.activation(rsu[:], avs[u][64:65, :], AF.Copy)
                    rcp = rpool.tile([1, S], f32, tag="row", name=f"rc{j}_{u}")
                    nc.vector.reciprocal_approx_fast(rcp[:], rsu[:])
                    rcps.append(rcp)
                for u in range(2):
                    rb = rbpool.tile([64, S], f32, tag="rb", name=f"rb{j}_{u}")
                    nc.gpsimd.partition_broadcast(rb[:], rcps[u][:])
                    rbs.append(rb)
                for u in range(2):
                    nc.vector.tensor_tensor(
                        attnT[u * 64 : u * 64 + 64, j, :],
                        avs[u][0:64, :],
                        rbs[u][:],
                        OP.mult,
                    )

            def attention_core(qT, kT, vt, attnT, causal, mask_sb, fillers=()):
                """Stagger-2 software pipeline over the 4 head pairs: scores
                for pair j+2 are emitted between AV(j) blocks, so the PE queue
                always holds score matmuls while ACT runs exps, but PSUM/exp
                tile demand stays bounded (no scheduling cycle).
                `fillers` are callables emitting independent PE work (e.g.
                cross-K/V projection chunks) drained one per pair."""
                exs_all = [None] * NSUB
                exs_all[0] = attn_pair_scores(0, qT, kT, causal, mask_sb)
                exs_all[1] = attn_pair_scores(1, qT, kT, causal, mask_sb)
                for j in range(NSUB):
                    if j < len(fillers):
                        fillers[j]()
                    attn_av_pair(j, exs_all[j], vt, attnT, causal)
                    if j + 2 < NSUB:
                        exs_all[j + 2] = attn_pair_scores(
                            j + 2, qT, kT, causal, mask_sb
                        )

            def layer_norm(x_in, g_sb, b_sb, out_f32=False, fillers=(),
                           keep_warm=False):
                """LN over partitions (d), per-token stats via PE sums.
                rsd = exp(-0.5*ln(var)) keeps ACT in one table set.

                fillers[0] is emitted between the mean and s2 stat matmuls
                (PE otherwise stalls on the DVE squares), fillers[1] between
                the mb and sdb broadcasts (PE otherwise stalls ~3us on the
                mean->var->ln->exp row chain). keep_warm adds two tiny
                row-dependent matmuls in those slots instead, so HAM never
                sees a full idle window during the chain."""
                mean_ps = ps_tile(p=1)
                s2_ps = ps_tile(p=1)
                sq = tpool.tile([P, NSUB, S], fsb, tag="lnsq", name="sq")
                for i in range(NSUB):
                    nc.gpsimd.tensor_tensor(
                        sq[:, i, :], x_in[:, i, :], x_in[:, i, :], OP.mult
                    )
                for i in range(NSUB):
                    mm(
                        mean_ps[:],
                        invD_sb[:],
                        x_in[:, i, :],
                        start=(i == 0),
                        stop=(i == NSUB - 1),
                    )
                if len(fillers) > 0:
                    fillers[0]()
                for i in range(NSUB):
                    mm(
                        s2_ps[:],
                        invD_sb[:],
                        sq[:, i, :],
                        start=(i == 0),
                        stop=(i == NSUB - 1),
                    )
                mean_sb = rpool.tile([1, S], fsb, tag="row", name="mean")
                nc.scalar.activation(mean_sb[:], mean_ps[:], AF.Copy)
                # bf16 is enough for mean^2 (means are ~0 post-residual) and
                # lets the keep_warm matmul use it as a stationary operand
                msq_sb = rpool.tile([1, S], fsb, tag="row", name="msq")
                nc.vector.tensor_tensor(msq_sb[:], mean_ps[:], mean_sb[:], OP.mult)
                # var = (s2 + eps) - mean^2, fused
                var_sb = rpool.tile([1, S], f32, tag="row", name="var")
                nc.vector.scalar_tensor_tensor(
                    var_sb[:], s2_ps[:], float(EPS), msq_sb[:], OP.add, OP.subtract
                )
                lnv_sb = rpool.tile([1, S], f32, tag="row", name="lnv")
                nc.scalar.activation(lnv_sb[:], var_sb[:], AF.Ln)
                rsd = rpool.tile([1, S], fsb, tag="row", name="rsd")
                nc.scalar.activation(rsd[:], lnv_sb[:], AF.Exp, scale=-0.5)
                mb_ps = ps_tile()
                mm(mb_ps[:], ones_sb[0:1, 0:P], mean_sb[0:1, :], start=True, stop=True)
                if len(fillers) > 1:
                    fillers[1]()
                elif keep_warm:
                    # touch the PE mid-chain (gated on msq, ~halfway through
                    # the row chain) so the HAM idle window resets
                    warm_ps = ps_tile()
                    mm(warm_ps[:], msq_sb[0:1, 0:P], ones_sb[0:1, :],
                       start=True, stop=True)
                sdb_ps = ps_tile()
                mm(sdb_ps[:], ones_sb[0:1, 0:P], rsd[0:1, :], start=True, stop=True)
                xo = (xopool if out_f32 else xpool).tile(
                    [P, NSUB, S],
                    f32 if out_f32 else fsb,
                    tag="xo" if out_f32 else "x",
                )
                for i in range(NSUB):
                    if unit_ln:
                        t1 = tpool.tile([P, S], fsb, tag="lnt")
                        nc.vector.tensor_tensor(
                            t1[:], x_in[:, i, :], mb_ps[:], OP.subtract
                        )
                        nc.vector.tensor_tensor(
                            xo[:, i, :], t1[:], sdb_ps[:], OP.mult
                        )
                    else:
                        t1 = tpool.tile([P, S], f32, tag="lnt")
                        nc.vector.tensor_tensor(
                            t1[:], x_in[:, i, :], mb_ps[:], OP.subtract
                        )
                        nc.vector.tensor_tensor(t1[:], t1[:], sdb_ps[:], OP.mult)
                        nc.scalar.activation(
                            xo[:, i, :],
                            t1[:],
                            AF.Identity,
                            bias=b_sb[:, i : i + 1],
                            scale=g_sb[:, i : i + 1],
                        )
                return xo

            def residual(a_T, b_T):
                # GPSIMD: both operands SBUF, and the engine is ~85% idle;
                # keeps the adds off the DVE FIFO which gates the LN chain
                xo = xpool.tile([P, NSUB, S], fsb, tag="x")
                for i in range(NSUB):
                    nc.gpsimd.tensor_tensor(
                        xo[:, i, :], a_T[:, i, :], b_T[:, i, :], OP.add
                    )
                return xo

            def load_w(src, l, shape):
                t = wpool.tile(shape, fsb, tag="wt")
                nc.sync.dma_start(t[:], src[l])
                return t

            def load_small(src, l, shape, tag):
                if tag == "brow":
                    t = brpool.tile(shape, fsb, tag=tag)
                else:
                    t = spool.tile(shape, f32, tag=tag)
                nc.sync.dma_start(t[:], src[l])
                return t

            def tap(name, tile_):
                if name in tap_d:
                    nc.sync.dma_start(tap_d[name][:], tile_[:])

            vc_cur = None
            for l in range(n_layers):
                # ---- self attention ----
                wq = load_w(wq_s_d, l, [P, NSUB, D])
                wk = load_w(wk_s_d, l, [P, NSUB, D])
                wv = load_w(wv_s_d, l, [P, NSUB, HW_COLS])
                if zb_attn:
                    bq = bk = bv = None
                else:
                    bq = load_small(bq_s_d, l, [P, NSUB], "bcol")
                    bk = load_small(bk_s_d, l, [P, NSUB], "bcol")
                    bv = load_small(bv_s_d, l, [1, HW_COLS], "brow")
                # chunks 0,1 of q/k up front (scores pairs 0,1 need them);
                # chunks 2,3 deferred into the AV phase as PE filler
                qT = proj_T(wq, bq, xT, on_act=True, chunks=(0, 1))
                kT = proj_T(wk, bk, xT, on_act=False, chunks=(0, 1))
                vt = v_aug(wv, bv, xT)

                wkc = load_w(wk_c_d, l, [P, NSUB, D])
                bkc = None if zb_attn else load_small(bk_c_d, l, [P, NSUB], "bcol")
                if vc_cur is None:  # first layer computes its own cross-V
                    wvc0 = load_w(wv_c_d, l, [P, NSUB, HW_COLS])
                    bvc0 = (
                        None if zb_attn
                        else load_small(bv_c_d, l, [1, HW_COLS], "brow")
                    )

                # kcT chunks are spread over AV-phase and LN1 filler slots
                kcT = qkpool.tile([P, NSUB, S], fsb, tag="qk")

                def kc_chunk(j):
                    ps = ps_tile()
                    for i in range(NSUB):
                        mm(
                            ps[:],
                            wkc[:, i, j * P : (j + 1) * P],
                            encT[:, i, :],
                            start=(i == 0),
                            stop=(i == NSUB - 1),
                        )
                    if bkc is None:
                        nc.vector.tensor_copy(kcT[:, j, :], ps[:])
                    else:
                        nc.vector.tensor_scalar(
                            kcT[:, j, :], ps[:], bkc[:, j : j + 1], None, OP.add
                        )

                saT = apool.tile([P, NSUB, S], fsb, tag="attn")
                attention_core(
                    qT, kT, vt, saT, causal_self, smask_sb,
                    fillers=[
                        lambda: (
                            proj_chunk(qT, wq, bq, xT, 2, True),
                            proj_chunk(kT, wk, bk, xT, 2, False),
                        ),
                        lambda: (
                            proj_chunk(qT, wq, bq, xT, 3, True),
                            proj_chunk(kT, wk, bk, xT, 3, False),
                        ),
                        lambda: kc_chunk(0),
                        lambda: kc_chunk(1),
                    ],
                )
                if vc_cur is None:
                    vc_cur = v_aug(wvc0, bvc0, encT)
                tap(f"sa{l}", saT)

                if unit_ln:
                    g1 = b1c_ln = None
                else:
                    g1 = load_small(ln1g_d, l, [P, NSUB], "bcol")
                    b1c_ln = load_small(ln1b_d, l, [P, NSUB], "bcol")
                x1 = layer_norm(
                    residual(xT, saT), g1, b1c_ln,
                    fillers=[
                        lambda: kc_chunk(2),
                        lambda: kc_chunk(3),
                    ],
                )
                tap(f"x1_{l}", x1)

                # ---- cross attention ----
                wqc = load_w(wq_c_d, l, [P, NSUB, D])
                bqc = None if zb_attn else load_small(bq_c_d, l, [P, NSUB], "bcol")
                qcT = proj_T(wqc, bqc, x1, on_act=True, chunks=(0, 1))
                # next layer's cross-V projection fills the cross AV phase
                if l + 1 < n_layers:
                    wvc_n = load_w(wv_c_d, l + 1, [P, NSUB, HW_COLS])
                    bvc_n = (
                        None if zb_attn
                        else load_small(bv_c_d, l + 1, [1, HW_COLS], "brow")
                    )
                    vc_next = vpool.tile([P, NCH, H, DVh + 1], fsb, tag="v")
                    vx_fill = [
                        lambda: v_aug_into(vc_next, wvc_n, bvc_n, encT, (0, 1)),
                        lambda: v_aug_into(vc_next, wvc_n, bvc_n, encT, (2, 3)),
                    ]
                else:
                    vc_next = None
                    vx_fill = []
                caT = apool.tile([P, NSUB, S], fsb, tag="attn")
                attention_core(
                    qcT, kcT, vc_cur, caT, False, cmask_sb,
                    fillers=[
                        lambda: proj_chunk(qcT, wqc, bqc, x1, 2, True),
                        lambda: proj_chunk(qcT, wqc, bqc, x1, 3, True),
                    ] + vx_fill,
                )
                tap(f"ca{l}", caT)
                if unit_ln:
                    g2 = b2c_ln = None
                else:
                    g2 = load_small(ln2g_d, l, [P, NSUB], "bcol")
                    b2c_ln = load_small(ln2b_d, l, [P, NSUB], "bcol")
                x2 = layer_norm(
                    residual(x1, caT), g2, b2c_ln, keep_warm=True,
                )
                vc_cur = vc_next
                tap(f"x2_{l}", x2)

                # ---- FFN ----
                b1col = None if zb_ffn else load_small(b1_d, l, [P, NF], "b1col")
                ff1 = ffpool.tile([P, NF, S], fsb, tag="ff1")
                for g in range(4):  # w1 granules of 512 dff cols
                    w1g = wpool.tile([P, NSUB, 512], fsb, tag="wt")
                    nc.sync.dma_start(
                        w1g[:], w1_d[l, :, :, g * 512 : (g + 1) * 512]
                    )
                    for fl in range(4):
                        F = g * 4 + fl
                        ps = ps_tile()
                        for i in range(NSUB):
                            mm(
                                ps[:],
                                w1g[:, i, fl * P : (fl + 1) * P],
                                x2[:, i, :],
                                start=(i == 0),
                                stop=(i == NSUB - 1),
                            )
                        if b1col is None:
                            nc.scalar.activation(ff1[:, F, :], ps[:], AF.Relu)
                        else:
                            nc.scalar.activation(
                                ff1[:, F, :], ps[:], AF.Relu,
                                bias=b1col[:, F : F + 1],
                            )
                b2col = None if zb_ffn else load_small(b2_d, l, [P, NSUB], "bcol")
                ffo = apool.tile([P, NSUB, S], fsb, tag="attn")
                for j in range(NSUB):
                    w2g = wpool.tile([P, NF, P], fsb, tag="wt")
                    nc.sync.dma_start(w2g[:], w2_d[l, :, :, j * P : (j + 1) * P])
                    ps = ps_tile()
                    for k in range(NF):
                        mm(
                            ps[:],
                            w2g[:, k, :],
                            ff1[:, k, :],
                            start=(k == 0),
                            stop=(k == NF - 1),
                        )
                    if b2col is None:
                        nc.scalar.activation(ffo[:, j, :], ps[:], AF.Identity)
                    else:
                        nc.scalar.activation(
                            ffo[:, j, :], ps[:], AF.Identity,
                            bias=b2col[:, j : j + 1],
                        )
                tap(f"ff{l}", ffo)
                xT = layer_norm(
                    residual(x2, ffo), g2, b2c_ln,
                    out_f32=(l == n_layers - 1), keep_warm=True,
                )

            nc.sync.dma_start(out_d[:], xT[:])

    try:
        nc.compile()
    finally:
        _undo_act_patch()
    return nc


def _prep_shared(inputs, n_layers, zb_attn, zb_ffn, unit_ln):
    """Host-side marshalling of weights into device tile layouts (float32;
    kernel() casts matmul-side arrays to the MM_DT numpy dtype)."""
    g = {}
    emb = np.asarray(inputs["emb"], np.float32)

    def wqk_prep(w):  # [NL, H, D, DK] -> [nl, P, NSUB, D]
        out = np.empty((n_layers, P, NSUB, D), np.float32)
        for l in range(n_layers):
            w2d = np.asarray(w[l], np.float32).transpose(1, 0, 2).reshape(D, H * DK)
            out[l] = w2d.reshape(NSUB, P, H * DK).transpose(1, 0, 2)
        return np.ascontiguousarray(out)

    def wv_prep(w, bv):  # augmented: per head 64 v-cols + ones col
        wout = np.empty((n_layers, P, NSUB, HW_COLS), np.float32)
        brow = np.zeros((n_layers, 1, HW_COLS), np.float32)
        for l in range(n_layers):
            aug = np.zeros((D, HW_COLS), np.float32)
            baug = np.zeros(HW_COLS, np.float32)
            wl = np.asarray(w[l], np.float32)  # [H, D, DVh]
            bl = np.asarray(bv[l], np.float32)  # [H, DVh]
            for h in range(H):
                aug[:, h * 65 : h * 65 + 64] = wl[h]
                baug[h * 65 : h * 65 + 64] = bl[h]
                baug[h * 65 + 64] = 1.0
            wout[l] = aug.reshape(NSUB, P, HW_COLS).transpose(1, 0, 2)
            brow[l, 0] = baug
        return np.ascontiguousarray(wout), brow

    def bcol_prep(b):  # [NL, ...] -> [nl, P, width]
        out = np.stack(
            [_col_layout(np.asarray(b[l], np.float32)) for l in range(n_layers)]
        )
        return np.ascontiguousarray(out)

    g["wq_s"] = wqk_prep(inputs["Wq_s"])
    g["wk_s"] = wqk_prep(inputs["Wk_s"])
    g["wv_s"], bv_s = wv_prep(inputs["Wv_s"], inputs["bv_s"])
    g["wq_c"] = wqk_prep(inputs["Wq_c"])
    g["wk_c"] = wqk_prep(inputs["Wk_c"])
    g["wv_c"], bv_c = wv_prep(inputs["Wv_c"], inputs["bv_c"])
    if not zb_attn:
        g["bv_s"], g["bv_c"] = bv_s, bv_c
        g["bq_s"] = bcol_prep(inputs["bq_s"])
        g["bk_s"] = bcol_prep(inputs["bk_s"])
        g["bq_c"] = bcol_prep(inputs["bq_c"])
        g["bk_c"] = bcol_prep(inputs["bk_c"])

    w1 = np.empty((n_layers, P, NSUB, DFF), np.float32)
    w2 = np.empty((n_layers, P, NF, D), np.float32)
    for l in range(n_layers):
        w1[l] = (
            np.asarray(inputs["W1"][l], np.float32)
            .reshape(NSUB, P, DFF)
            .transpose(1, 0, 2)
        )
        w2[l] = (
            np.asarray(inputs["W2"][l], np.float32)
            .reshape(NF, P, D)
            .transpose(1, 0, 2)
        )
    g["w1"] = np.ascontiguousarray(w1)
    g["w2"] = np.ascontiguousarray(w2)
    if not zb_ffn:
        g["b1c"] = bcol_prep(inputs["b1"])
        g["b2c"] = bcol_prep(inputs["b2"])
    if not unit_ln:
        g["ln1g"] = bcol_prep(inputs["ln1_g"])
        g["ln1b"] = bcol_prep(inputs["ln1_b"])
        g["ln2g"] = bcol_prep(inputs["ln2_g"])
        g["ln2b"] = bcol_prep(inputs["ln2_b"])

    g["peT"] = _to_T_tiles(_pe_table())
    g["ones_row"] = np.ones((1, S), np.float32)
    g["invD_col"] = np.full((P, 1), 1.0 / D, np.float32)
    q = np.arange(P)
    g["tri01"] = (q[None, :] >= q[:, None]).astype(np.float32)
    g["ident"] = np.eye(P, dtype=np.float32)
    return g, emb


def _mask_T8(mask_b):
    """[S, S] additive mask -> [P, NCH, S] transposed, pre-scaled by 8."""
    m = np.ascontiguousarray(np.asarray(mask_b, np.float32).T) * 8.0
    return np.ascontiguousarray(m.reshape(NCH, P, S).transpose(1, 0, 2))


# f32 bias-column tensors; everything else carries the matmul dtype
_F32_KEYS = {
    "bq_s", "bk_s", "bq_c", "bk_c", "b1c", "b2c",
    "ln1g", "ln1b", "ln2g", "ln2b",
}


def _allzero(x):
    return bool(np.all(np.asarray(x) == 0.0))


def kernel(**inputs):
    global LAST_RESULT
    _ensure_path()
    import ml_dtypes
    from concourse.bass_utils import run_bass_kernel_spmd

    n_layers = N_LAYERS
    mm_np = np.float32 if MM_DT == "f32r" else ml_dtypes.bfloat16
    ids = np.asarray(inputs["decoder_input"])
    enc = np.asarray(inputs["encoder_output"], np.float32)
    smask = np.asarray(inputs["self_mask"], np.float32)
    cmask = np.asarray(inputs["cross_mask"], np.float32)

    tril = np.tril(np.ones((S, S), bool))
    canon = np.where(tril, np.float32(0.0), np.float32(-1e9))
    causal_self = all(np.array_equal(smask[b], canon) for b in range(B))
    self_needs_mask = (not causal_self) and bool(np.any(smask != 0.0))
    cross_needs_mask = bool(np.any(cmask != 0.0))

    zb_attn = all(
        _allzero(inputs[k])
        for k in ("bq_s", "bk_s", "bv_s", "bq_c", "bk_c", "bv_c")
    )
    zb_ffn = _allzero(inputs["b1"]) and _allzero(inputs["b2"])
    unit_ln = (
        _allzero(inputs["ln1_b"]) and _allzero(inputs["ln2_b"])
        and bool(np.all(np.asarray(inputs["ln1_g"]) == 1.0))
        and bool(np.all(np.asarray(inputs["ln2_g"]) == 1.0))
    )

    shared, emb = _prep_shared(inputs, n_layers, zb_attn, zb_ffn, unit_ln)
    shared = {
        k: (v if k in _F32_KEYS else v.astype(mm_np)) for k, v in shared.items()
    }

    key = (n_layers, causal_self, self_needs_mask, cross_needs_mask,
           zb_attn, zb_ffn, unit_ln, tuple(TAPS), MM_DT)
    if key not in _BUILD_CACHE:
        _BUILD_CACHE[key] = _build(
            n_layers, causal_self, self_needs_mask, cross_needs_mask,
            zb_attn, zb_ffn, unit_ln, tuple(TAPS), MM_DT,
        )
    nc = _BUILD_CACHE[key]

    in_maps = []
    for b in range(B):
        m = dict(shared)
        m["x0T"] = _to_T_tiles(emb[ids[b]]).astype(mm_np)
        m["encT"] = _to_T_tiles(enc[b]).astype(mm_np)
        if self_needs_mask:
            m["smaskT8"] = _mask_T8(smask[b]).astype(mm_np)
        if cross_needs_mask:
            m["cmaskT8"] = _mask_T8(cmask[b]).astype(mm_np)
        if not causal_self:
            m.pop("tri01", None)
        if not (self_needs_mask or cross_needs_mask):
            m.pop("ident", None)
        in_maps.append(m)

    res = run_bass_kernel_spmd(nc, in_maps, core_ids=list(range(8)))
    LAST_RESULT = res

    out = np.empty((B, S, D), np.float32)
    for b in range(B):
        xt = np.asarray(res.results[b]["out_xT"], np.float32)  # [P, NSUB, S]
        out[b] = xt.transpose(1, 0, 2).reshape(D, S).T
    return out
